# revision 30
# baseline (speedup 1.0000x reference)
"""Multi-head attention (B=2, N=2048, dim=1024, heads=16, dim_head=64) on
8 TRN2 NeuronCores.

Sharding: data-parallel over batch (2) x tensor-parallel over heads (4 per
core).  Core c handles batch b = c//4 and heads [4g, 4g+4), g = c%4.  Each
core computes its 4 heads' attention plus the partial out-projection
(O_heads @ w_out[head rows]); the host sums the 4 partials per batch and
adds the bias.

Per-core device algorithm (all matmuls in float32r, fp32 PSUM accumulate):
  xT  [1024, 2048] = x[b].T               (transposed on host, free)
  Qt/Kt [128(i of head pair), 2048(n)] = w_slice.T @ xT    (W stationary)
  V   [128(m), 16(mt), 4(h), 65]  natural layout, col 64 = ones so the
       attention-value matmul also produces the softmax denominator.
  per head pair (A,B packed in PE rows 0-63 / 64-127 via tile_position),
  per 512-wide query chunk, per key tile mt:
       St = Kt_h[64, 128].T @ Qt_h[64, 512]      -> PSUM [128(m), 512(n)]
       Pt = exp(St * 1/8)                        -> SBUF (ScalarE, fused scale)
       Ot' += V'[128, 65].T @ Pt                 -> PSUM [65, 512] accum
  normalize: o = Ot'[0:64] * (1 / Ot'[64]) (DVE recip + GPSIMD bcast + DVE)
  proj: out[nt*128:+128, jc*512:+512] = sum_p o_sb[:,p,nt].T @ wo[:,p,jc]
"""
import numpy as np

import concourse.bass as bass
import concourse.mybir as mybir
import concourse.tile as tile
from concourse import bacc
from concourse.bass_utils import run_bass_kernel_spmd

# Problem constants (hardcoded per contract).
B = 2
N = 2048
DIM = 1024
HEADS = 16
DH = 64
INNER = HEADS * DH
SCALE = DH ** -0.5

N_CORES = 8
HEADS_PER_CORE = 4
PAIRS = 2          # head pairs per core
NT = N // 128      # 16 key/query tiles
DT = DIM // 128    # 8 contraction tiles
CH = N // 512      # 4 query chunks
F32 = mybir.dt.float32
F32R = mybir.dt.float32r
BF16 = mybir.dt.bfloat16

_CACHED_NC = None


def _emit_kernel(tc, xt_d, wqkv_d, wo_d, out_d):
    nc = tc.nc

    from contextlib import ExitStack

    ctx = ExitStack()
    per = ctx.enter_context(tc.tile_pool(name="persist", bufs=1))
    psum = ctx.enter_context(tc.tile_pool(name="psum", bufs=1, space="PSUM"))
    work = ctx.enter_context(tc.tile_pool(name="work", bufs=1))

    # Persistent SBUF tensors.
    xt_sb = per.tile([128, DT, N], BF16, tag="xt")
    wqkv_sb = per.tile([128, DT, 768], BF16, tag="wqkv")
    wo_sb = per.tile([128, PAIRS, DIM], BF16, tag="wo")
    qt_sb = per.tile([128, PAIRS, N], BF16, tag="qt")
    kt_sb = per.tile([128, PAIRS, N], BF16, tag="kt")
    v_sb = per.tile([128, NT, HEADS_PER_CORE, DH + 1], BF16, tag="v")
    o_sb = per.tile([128, PAIRS, N], BF16, tag="o")

    # Input DMAs: weights first (small, needed immediately), then xT by
    # chunk so compute can start after ~2 MB.
    for dt in range(DT):
        nc.sync.dma_start(wqkv_sb[:, dt, :], wqkv_d[128 * dt:128 * (dt + 1), :])
        nc.sync.dma_start(
            xt_sb[:, dt, 0:512], xt_d[128 * dt:128 * (dt + 1), 0:512]
        )
    for c in range(1, CH):
        for dt in range(DT):
            nc.sync.dma_start(
                xt_sb[:, dt, 512 * c:512 * (c + 1)],
                xt_d[128 * dt:128 * (dt + 1), 512 * c:512 * (c + 1)],
            )
        if c == 1:
            for p in range(PAIRS):
                nc.sync.dma_start(wo_sb[:, p, :], wo_d[128 * p:128 * (p + 1), :])
    # Ones column of V' (gives the softmax denominator through the AV matmul).
    ones_sb = per.tile([128, NT * HEADS_PER_CORE], F32, tag="ones")
    nc.vector.memset(ones_sb[:], 1.0)
    nc.vector.tensor_copy(
        v_sb[:, :, :, DH:DH + 1],
        ones_sb[:].rearrange("p (a b c) -> p a b c", b=HEADS_PER_CORE, c=1),
    )
    # Touch Exp once so the ACT table DMA (~1.3us + pseudo-load) happens
    # during the startup phase rather than before the first real exp.
    warm = work.tile([1, 1], F32, tag="warm")
    nc.scalar.activation(
        warm[:], ones_sb[0:1, 0:1], mybir.ActivationFunctionType.Exp, scale=1.0
    )

    def emit_qk_chunk(which, p, c):
        """Qt or Kt for head pair p, n-chunk c: [128, 512] of W.T @ xT."""
        src = qt_sb if which == "q" else kt_sb
        col0 = (0 if which == "q" else 256) + 128 * p
        ps = psum.tile([128, 512], F32, tag="qk", bufs=2)
        for dt in range(DT):
            nc.tensor.matmul(
                ps[:],
                wqkv_sb[:, dt, col0:col0 + 128],
                xt_sb[:, dt, 512 * c:512 * (c + 1)],
                start=(dt == 0),
                stop=(dt == DT - 1),
            )
        nc.vector.tensor_copy(src[:, p, 512 * c:512 * (c + 1)], ps[:])

    def emit_v_tile(mt):
        """V natural [128(m), 256(4 heads x 64)] for key tile mt."""
        ps = psum.tile([128, 256], F32, tag="qk", bufs=2)
        for dt in range(DT):
            nc.tensor.matmul(
                ps[:],
                xt_sb[:, dt, 128 * mt:128 * (mt + 1)],
                wqkv_sb[:, dt, 512:768],
                start=(dt == 0),
                stop=(dt == DT - 1),
            )
        nc.vector.tensor_copy(
            v_sb[:, mt, :, 0:DH],
            ps[:].rearrange("p (h d) -> p h d", h=HEADS_PER_CORE),
        )

    def emit_att_chunk(p, c, filler=None):
        """Attention for head pair p, query chunk c (cols 512c..512c+512).

        The two heads of a pair sit in PE rows 0-63 / 64-127 (tile_position
        row packing) so their bf16 St matmuls run concurrently.
        """
        ot = [
            psum.tile([DH + 1, 512], F32, tag="ot", bufs=3, name=f"ot{h}")
            for h in range(2)
        ]
        # AV matmuls lag the St/exp of the current key tile so the in-order
        # PE queue never head-of-line blocks on ScalarE, and are flushed in
        # same-bank pairs (AVh(mt), AVh(mt+1) back-to-back) — consecutive
        # matmuls into the same PSUM bank avoid the bank-switch penalty.
        pending = []

        def flush(n):
            for h in range(2):
                for pmt, ppts in pending[:n]:
                    nc.tensor.matmul(
                        ot[h][:],
                        v_sb[:, pmt, 2 * p + h, :],
                        ppts[h][:],
                        start=(pmt == 0),
                        stop=(pmt == NT - 1),
                    )
            del pending[:n]

        for mt in range(NT):
            st = [None, None]
            for h in range(2):
                st[h] = psum.tile([128, 512], F32, tag="st", bufs=3, name=f"st{h}")
                nc.tensor.matmul(
                    st[h][:],
                    kt_sb[64 * h:64 * (h + 1), p, 128 * mt:128 * (mt + 1)],
                    qt_sb[64 * h:64 * (h + 1), p, 512 * c:512 * (c + 1)],
                    start=True,
                    stop=True,
                    tile_position=(64 * h, 0),
                )
            pts = [None, None]
            for h in range(2):
                pts[h] = work.tile([128, 512], BF16, tag="pt", bufs=8, name=f"pt{h}")
                nc.scalar.activation(
                    pts[h][:], st[h][:], mybir.ActivationFunctionType.Exp,
                    scale=SCALE,
                )
            pending.append((mt, pts))
            if len(pending) == 2:
                flush(1)
            if filler is not None:
                filler(c, mt)
        if filler is not None:
            filler(c, NT)  # hide the last exp's latency behind filler work
        flush(len(pending))
        # Normalize: o = Ot'[0:64] / Ot'[64].  Interleave the two heads so
        # the DVE/GPSIMD stages pipeline instead of serializing.
        den, recip, rbc = [None, None], [None, None], [None, None]
        for h in range(2):
            den[h] = work.tile([1, 512], F32, tag="den", bufs=4, name=f"den{h}")
            nc.vector.tensor_copy(den[h][:], ot[h][DH:DH + 1, :])
        for h in range(2):
            recip[h] = work.tile([1, 512], F32, tag="recip", bufs=4, name=f"rec{h}")
            nc.vector.reciprocal_approx_fast(recip[h][:], den[h][:])
        for h in range(2):
            rbc[h] = work.tile([64, 512], F32, tag="rbc", bufs=4, name=f"rbc{h}")
            nc.gpsimd.partition_broadcast(rbc[h][:], recip[h][:])
        for h in range(2):
            nc.vector.tensor_mul(
                o_sb[64 * h:64 * (h + 1), p, 512 * c:512 * (c + 1)],
                ot[h][0:DH, :],
                rbc[h][:],
            )

    def emit_proj_unit(nt, jc):
        """out[128nt:+128, 512jc:+512] = sum_p o_sb[:,p,nt].T @ wo[:,p,jc]."""
        ps = psum.tile([128, 512], F32, tag="qk", bufs=2)
        for p in range(PAIRS):
            nc.tensor.matmul(
                ps[:],
                o_sb[:, p, 128 * nt:128 * (nt + 1)],
                wo_sb[:, p, 512 * jc:512 * (jc + 1)],
                start=(p == 0),
                stop=(p == PAIRS - 1),
            )
        ev = work.tile([128, 512], F32, tag="ev", bufs=4)
        nc.vector.tensor_copy(ev[:], ps[:])
        nc.sync.dma_start(
            out_d[128 * nt:128 * (nt + 1), 512 * jc:512 * (jc + 1)], ev[:]
        )

    # ---- Emission schedule ----
    # Phase 0: the minimum needed for the first St: Kt pair-0 (all m),
    # Qt pair-0 chunk 0, first few V tiles.  Everything else streams in as
    # filler work inside the attention loops to keep the PE dense (HAM warm)
    # while ScalarE chews exps.
    for c in range(CH):
        emit_qk_chunk("k", 0, c)
    emit_qk_chunk("q", 0, 0)
    for mt in range(4):
        emit_v_tile(mt)

    # Fillers for attention pair 0, per chunk.  V tiles stream 4 mt ahead of
    # their consumer during chunk 0; the Qt chunk for c+1 is emitted near the
    # end of chunk c so it completes just in time.
    att0_fill = {
        0: [("v", mt) for mt in range(4, NT)] + [("qk", "q", 0, 1)],
        1: [("qk", "q", 0, 2), ("qk", "k", 1, 0), ("qk", "k", 1, 1),
            ("qk", "k", 1, 2)],
        2: [("qk", "q", 0, 3), ("qk", "k", 1, 3), ("qk", "q", 1, 0),
            ("qk", "q", 1, 1)],
        3: [("qk", "q", 1, 2), ("qk", "q", 1, 3)],
    }

    def att0_filler(c, mt):
        q = att0_fill[c]
        if not q:
            return
        if c == 0:
            unit = q.pop(0)
            emit_v_tile(unit[1]) if unit[0] == "v" else emit_qk_chunk(
                unit[1], unit[2], unit[3]
            )
        elif mt % 2 == 1 or mt >= NT - 1:
            _, which, fp, fc = q.pop(0)
            emit_qk_chunk(which, fp, fc)

    for c in range(CH):
        emit_att_chunk(0, c, filler=att0_filler)
    for c in range(CH):
        for unit in att0_fill[c]:
            if unit[0] == "v":
                emit_v_tile(unit[1])
            else:
                emit_qk_chunk(unit[1], unit[2], unit[3])
        att0_fill[c] = []

    # Phase 2: attention pair 1; proj units for chunk c-1 fill chunk c's
    # exp latency (chunk c-1's O rows are complete for both pairs).
    proj_q = []

    def att1_filler(c, mt):
        if (mt % 2 == 1 or mt >= NT - 1) and proj_q:
            nt, jc = proj_q.pop(0)
            emit_proj_unit(nt, jc)

    for c in range(CH):
        emit_att_chunk(1, c, filler=att1_filler)
        proj_q += [(nt, jc) for nt in range(4 * c, 4 * c + 4) for jc in range(2)]
    while proj_q:
        nt, jc = proj_q.pop(0)
        emit_proj_unit(nt, jc)

    ctx.close()


def _build():
    global _CACHED_NC
    if _CACHED_NC is not None:
        return _CACHED_NC
    nc = bacc.Bacc(
        "TRN2",
        target_bir_lowering=False,
        debug=False,
        enable_asserts=True,
        num_devices=N_CORES,
    )
    xt_d = nc.dram_tensor("xt", [DIM, N], BF16, kind="ExternalInput").ap()
    wqkv_d = nc.dram_tensor("wqkv", [DIM, 768], BF16, kind="ExternalInput").ap()
    wo_d = nc.dram_tensor("wo", [256, DIM], BF16, kind="ExternalInput").ap()
    out_d = nc.dram_tensor("out", [N, DIM], F32, kind="ExternalOutput").ap()

    with tile.TileContext(nc) as tc:
        _emit_kernel(tc, xt_d, wqkv_d, wo_d, out_d)
    nc.compile()
    _CACHED_NC = nc
    return nc


def _in_maps(x, w_qkv, w_out):
    import ml_dtypes

    bf = ml_dtypes.bfloat16
    maps = []
    for c in range(N_CORES):
        b, g = divmod(c, 4)
        cols = slice(256 * g, 256 * (g + 1))
        wqkv_c = np.ascontiguousarray(
            np.concatenate(
                [
                    w_qkv[:, cols],
                    w_qkv[:, INNER:][:, cols],
                    w_qkv[:, 2 * INNER:][:, cols],
                ],
                axis=1,
            ).astype(bf)
        )
        maps.append(
            {
                "xt": np.ascontiguousarray(x[b].T.astype(bf)),
                "wqkv": wqkv_c,
                "wo": np.ascontiguousarray(w_out[cols, :].astype(bf)),
            }
        )
    return maps


def _run(x, w_qkv, w_out, b_out, trace=False):
    nc = _build()
    res = run_bass_kernel_spmd(
        nc, _in_maps(x, w_qkv, w_out), list(range(N_CORES)), trace=trace
    )
    partials = np.stack([res.results[c]["out"] for c in range(N_CORES)])
    out = np.empty((B, N, DIM), dtype=np.float32)
    for b in range(B):
        out[b] = partials[4 * b:4 * b + 4].sum(axis=0) + b_out
    return out, res


def kernel(x, w_qkv, w_out, b_out):
    out, _ = _run(
        np.asarray(x, dtype=np.float32),
        np.asarray(w_qkv, dtype=np.float32),
        np.asarray(w_out, dtype=np.float32),
        np.asarray(b_out, dtype=np.float32),
    )
    return out


# revision 31
# speedup vs baseline: 1.0240x; 1.0240x over previous
"""Multi-head attention (B=2, N=2048, dim=1024, heads=16, dim_head=64) on
8 TRN2 NeuronCores.

Sharding: data-parallel over batch (2) x tensor-parallel over heads (4 per
core).  Core c handles batch b = c//4 and heads [4g, 4g+4), g = c%4.  Each
core computes its 4 heads' attention plus the partial out-projection
(O_heads @ w_out[head rows]); the host sums the 4 partials per batch and
adds the bias.

Per-core device algorithm (all matmuls in float32r, fp32 PSUM accumulate):
  xT  [1024, 2048] = x[b].T               (transposed on host, free)
  Qt/Kt [128(i of head pair), 2048(n)] = w_slice.T @ xT    (W stationary)
  V   [128(m), 16(mt), 4(h), 65]  natural layout, col 64 = ones so the
       attention-value matmul also produces the softmax denominator.
  per head pair (A,B packed in PE rows 0-63 / 64-127 via tile_position),
  per 512-wide query chunk, per key tile mt:
       St = Kt_h[64, 128].T @ Qt_h[64, 512]      -> PSUM [128(m), 512(n)]
       Pt = exp(St * 1/8)                        -> SBUF (ScalarE, fused scale)
       Ot' += V'[128, 65].T @ Pt                 -> PSUM [65, 512] accum
  normalize: o = Ot'[0:64] * (1 / Ot'[64]) (DVE recip + GPSIMD bcast + DVE)
  proj: out[nt*128:+128, jc*512:+512] = sum_p o_sb[:,p,nt].T @ wo[:,p,jc]
"""
import numpy as np

import concourse.bass as bass
import concourse.mybir as mybir
import concourse.tile as tile
from concourse import bacc
from concourse.bass_utils import run_bass_kernel_spmd

# Problem constants (hardcoded per contract).
B = 2
N = 2048
DIM = 1024
HEADS = 16
DH = 64
INNER = HEADS * DH
SCALE = DH ** -0.5

N_CORES = 8
HEADS_PER_CORE = 4
PAIRS = 2          # head pairs per core
NT = N // 128      # 16 key/query tiles
DT = DIM // 128    # 8 contraction tiles
CH = N // 512      # 4 query chunks
F32 = mybir.dt.float32
F32R = mybir.dt.float32r
BF16 = mybir.dt.bfloat16

_CACHED_NC = None


def _emit_kernel(tc, xt_d, wqkv_d, wo_d, out_d):
    nc = tc.nc

    from contextlib import ExitStack

    ctx = ExitStack()
    per = ctx.enter_context(tc.tile_pool(name="persist", bufs=1))
    psum = ctx.enter_context(tc.tile_pool(name="psum", bufs=1, space="PSUM"))
    work = ctx.enter_context(tc.tile_pool(name="work", bufs=1))

    # Persistent SBUF tensors.
    xt_sb = per.tile([128, DT, N], BF16, tag="xt")
    wqkv_sb = per.tile([128, DT, 768], BF16, tag="wqkv")
    wo_sb = per.tile([128, PAIRS, DIM], BF16, tag="wo")
    qt_sb = per.tile([128, PAIRS, N], BF16, tag="qt")
    kt_sb = per.tile([128, PAIRS, N], BF16, tag="kt")
    v_sb = per.tile([128, NT, HEADS_PER_CORE, DH + 1], BF16, tag="v")
    o_sb = per.tile([128, PAIRS, N], BF16, tag="o")

    # Input DMAs: weights first (small, needed immediately), then xT by
    # chunk so compute can start after ~2 MB.
    for dt in range(DT):
        nc.sync.dma_start(wqkv_sb[:, dt, :], wqkv_d[128 * dt:128 * (dt + 1), :])
        nc.sync.dma_start(
            xt_sb[:, dt, 0:512], xt_d[128 * dt:128 * (dt + 1), 0:512]
        )
    for c in range(1, CH):
        for dt in range(DT):
            nc.sync.dma_start(
                xt_sb[:, dt, 512 * c:512 * (c + 1)],
                xt_d[128 * dt:128 * (dt + 1), 512 * c:512 * (c + 1)],
            )
        if c == 1:
            for p in range(PAIRS):
                nc.sync.dma_start(wo_sb[:, p, :], wo_d[128 * p:128 * (p + 1), :])
    # Ones column of V' (gives the softmax denominator through the AV matmul).
    ones_sb = per.tile([128, NT * HEADS_PER_CORE], F32, tag="ones")
    nc.vector.memset(ones_sb[:], 1.0)
    nc.vector.tensor_copy(
        v_sb[:, :, :, DH:DH + 1],
        ones_sb[:].rearrange("p (a b c) -> p a b c", b=HEADS_PER_CORE, c=1),
    )
    # Touch Exp once so the ACT table DMA (~1.3us + pseudo-load) happens
    # during the startup phase rather than before the first real exp.
    warm = work.tile([1, 1], F32, tag="warm")
    nc.scalar.activation(
        warm[:], ones_sb[0:1, 0:1], mybir.ActivationFunctionType.Exp, scale=1.0
    )

    def emit_qk_chunk(which, p, c):
        """Qt or Kt for head pair p, n-chunk c: [128, 512] of W.T @ xT."""
        src = qt_sb if which == "q" else kt_sb
        col0 = (0 if which == "q" else 256) + 128 * p
        ps = psum.tile([128, 512], F32, tag="qk", bufs=2)
        for dt in range(DT):
            nc.tensor.matmul(
                ps[:],
                wqkv_sb[:, dt, col0:col0 + 128],
                xt_sb[:, dt, 512 * c:512 * (c + 1)],
                start=(dt == 0),
                stop=(dt == DT - 1),
            )
        nc.vector.tensor_copy(src[:, p, 512 * c:512 * (c + 1)], ps[:])

    def emit_v_tile(mt):
        """V natural [128(m), 256(4 heads x 64)] for key tile mt."""
        ps = psum.tile([128, 256], F32, tag="qk", bufs=2)
        for dt in range(DT):
            nc.tensor.matmul(
                ps[:],
                xt_sb[:, dt, 128 * mt:128 * (mt + 1)],
                wqkv_sb[:, dt, 512:768],
                start=(dt == 0),
                stop=(dt == DT - 1),
            )
        nc.vector.tensor_copy(
            v_sb[:, mt, :, 0:DH],
            ps[:].rearrange("p (h d) -> p h d", h=HEADS_PER_CORE),
        )

    def emit_att_chunk(p, c, filler=None):
        """Attention for head pair p, query chunk c (cols 512c..512c+512).

        The two heads of a pair sit in PE rows 0-63 / 64-127 (tile_position
        row packing) so their bf16 St matmuls run concurrently.
        """
        ot = [
            psum.tile([DH + 1, 512], F32, tag="ot", bufs=3, name=f"ot{h}")
            for h in range(2)
        ]
        # AV matmuls lag the St/exp of the current key tile so the in-order
        # PE queue never head-of-line blocks on ScalarE, and are flushed in
        # same-bank pairs (AVh(mt), AVh(mt+1) back-to-back) — consecutive
        # matmuls into the same PSUM bank avoid the bank-switch penalty.
        pending = []

        def flush(n):
            for h in range(2):
                for pmt, ppts in pending[:n]:
                    nc.tensor.matmul(
                        ot[h][:],
                        v_sb[:, pmt, 2 * p + h, :],
                        ppts[h][:],
                        start=(pmt == 0),
                        stop=(pmt == NT - 1),
                    )
            del pending[:n]

        for mt in range(NT):
            st = [None, None]
            for h in range(2):
                st[h] = psum.tile([128, 512], F32, tag="st", bufs=3, name=f"st{h}")
                nc.tensor.matmul(
                    st[h][:],
                    kt_sb[64 * h:64 * (h + 1), p, 128 * mt:128 * (mt + 1)],
                    qt_sb[64 * h:64 * (h + 1), p, 512 * c:512 * (c + 1)],
                    start=True,
                    stop=True,
                    tile_position=(64 * h, 0),
                )
            pts = [None, None]
            for h in range(2):
                pts[h] = work.tile([128, 512], BF16, tag="pt", bufs=12, name=f"pt{h}")
                nc.scalar.activation(
                    pts[h][:], st[h][:], mybir.ActivationFunctionType.Exp,
                    scale=SCALE,
                )
            pending.append((mt, pts))
            if len(pending) == 2:
                flush(1)
            if filler is not None:
                filler(c, mt)
        if filler is not None:
            filler(c, NT)  # hide the last exp's latency behind filler work
        flush(len(pending))
        # Normalize: o = Ot'[0:64] / Ot'[64].  Interleave the two heads so
        # the DVE/GPSIMD stages pipeline instead of serializing.
        den, recip, rbc = [None, None], [None, None], [None, None]
        for h in range(2):
            den[h] = work.tile([1, 512], F32, tag="den", bufs=4, name=f"den{h}")
            nc.vector.tensor_copy(den[h][:], ot[h][DH:DH + 1, :])
        for h in range(2):
            recip[h] = work.tile([1, 512], F32, tag="recip", bufs=4, name=f"rec{h}")
            nc.vector.reciprocal_approx_fast(recip[h][:], den[h][:])
        for h in range(2):
            rbc[h] = work.tile([64, 512], F32, tag="rbc", bufs=4, name=f"rbc{h}")
            nc.gpsimd.partition_broadcast(rbc[h][:], recip[h][:])
        for h in range(2):
            nc.vector.tensor_mul(
                o_sb[64 * h:64 * (h + 1), p, 512 * c:512 * (c + 1)],
                ot[h][0:DH, :],
                rbc[h][:],
            )

    def emit_proj_unit(nt, jc):
        """out[128nt:+128, 512jc:+512] = sum_p o_sb[:,p,nt].T @ wo[:,p,jc]."""
        ps = psum.tile([128, 512], F32, tag="qk", bufs=2)
        for p in range(PAIRS):
            nc.tensor.matmul(
                ps[:],
                o_sb[:, p, 128 * nt:128 * (nt + 1)],
                wo_sb[:, p, 512 * jc:512 * (jc + 1)],
                start=(p == 0),
                stop=(p == PAIRS - 1),
            )
        ev = work.tile([128, 512], F32, tag="ev", bufs=4)
        nc.vector.tensor_copy(ev[:], ps[:])
        nc.sync.dma_start(
            out_d[128 * nt:128 * (nt + 1), 512 * jc:512 * (jc + 1)], ev[:]
        )

    # ---- Emission schedule ----
    # Phase 0: the minimum needed for the first St: Kt pair-0 (all m),
    # Qt pair-0 chunk 0, first few V tiles.  Everything else streams in as
    # filler work inside the attention loops to keep the PE dense (HAM warm)
    # while ScalarE chews exps.
    for c in range(CH):
        emit_qk_chunk("k", 0, c)
    emit_qk_chunk("q", 0, 0)
    for mt in range(4):
        emit_v_tile(mt)

    # Fillers for attention pair 0, per chunk.  V tiles stream 4 mt ahead of
    # their consumer during chunk 0; the Qt chunk for c+1 is emitted near the
    # end of chunk c so it completes just in time.
    att0_fill = {
        0: [("v", mt) for mt in range(4, NT)] + [("qk", "q", 0, 1)],
        1: [("qk", "q", 0, 2), ("qk", "k", 1, 0), ("qk", "k", 1, 1),
            ("qk", "k", 1, 2)],
        2: [("qk", "q", 0, 3), ("qk", "k", 1, 3), ("qk", "q", 1, 0),
            ("qk", "q", 1, 1)],
        3: [("qk", "q", 1, 2), ("qk", "q", 1, 3)],
    }

    def att0_filler(c, mt):
        q = att0_fill[c]
        if not q:
            return
        if c == 0:
            unit = q.pop(0)
            emit_v_tile(unit[1]) if unit[0] == "v" else emit_qk_chunk(
                unit[1], unit[2], unit[3]
            )
        elif mt % 2 == 1 or mt >= NT - 1:
            _, which, fp, fc = q.pop(0)
            emit_qk_chunk(which, fp, fc)

    for c in range(CH):
        emit_att_chunk(0, c, filler=att0_filler)
    for c in range(CH):
        for unit in att0_fill[c]:
            if unit[0] == "v":
                emit_v_tile(unit[1])
            else:
                emit_qk_chunk(unit[1], unit[2], unit[3])
        att0_fill[c] = []

    # Phase 2: attention pair 1; proj units for chunk c-1 fill chunk c's
    # exp latency (chunk c-1's O rows are complete for both pairs).
    proj_q = []

    def att1_filler(c, mt):
        if (mt % 2 == 1 or mt >= NT - 1) and proj_q:
            nt, jc = proj_q.pop(0)
            emit_proj_unit(nt, jc)

    for c in range(CH):
        emit_att_chunk(1, c, filler=att1_filler)
        proj_q += [(nt, jc) for nt in range(4 * c, 4 * c + 4) for jc in range(2)]
    while proj_q:
        nt, jc = proj_q.pop(0)
        emit_proj_unit(nt, jc)

    ctx.close()


def _build():
    global _CACHED_NC
    if _CACHED_NC is not None:
        return _CACHED_NC
    nc = bacc.Bacc(
        "TRN2",
        target_bir_lowering=False,
        debug=False,
        enable_asserts=True,
        num_devices=N_CORES,
    )
    xt_d = nc.dram_tensor("xt", [DIM, N], BF16, kind="ExternalInput").ap()
    wqkv_d = nc.dram_tensor("wqkv", [DIM, 768], BF16, kind="ExternalInput").ap()
    wo_d = nc.dram_tensor("wo", [256, DIM], BF16, kind="ExternalInput").ap()
    out_d = nc.dram_tensor("out", [N, DIM], F32, kind="ExternalOutput").ap()

    with tile.TileContext(nc) as tc:
        _emit_kernel(tc, xt_d, wqkv_d, wo_d, out_d)
    nc.compile()
    _CACHED_NC = nc
    return nc


def _in_maps(x, w_qkv, w_out):
    import ml_dtypes

    bf = ml_dtypes.bfloat16
    maps = []
    for c in range(N_CORES):
        b, g = divmod(c, 4)
        cols = slice(256 * g, 256 * (g + 1))
        wqkv_c = np.ascontiguousarray(
            np.concatenate(
                [
                    w_qkv[:, cols],
                    w_qkv[:, INNER:][:, cols],
                    w_qkv[:, 2 * INNER:][:, cols],
                ],
                axis=1,
            ).astype(bf)
        )
        maps.append(
            {
                "xt": np.ascontiguousarray(x[b].T.astype(bf)),
                "wqkv": wqkv_c,
                "wo": np.ascontiguousarray(w_out[cols, :].astype(bf)),
            }
        )
    return maps


def _run(x, w_qkv, w_out, b_out, trace=False):
    nc = _build()
    res = run_bass_kernel_spmd(
        nc, _in_maps(x, w_qkv, w_out), list(range(N_CORES)), trace=trace
    )
    partials = np.stack([res.results[c]["out"] for c in range(N_CORES)])
    out = np.empty((B, N, DIM), dtype=np.float32)
    for b in range(B):
        out[b] = partials[4 * b:4 * b + 4].sum(axis=0) + b_out
    return out, res


def kernel(x, w_qkv, w_out, b_out):
    out, _ = _run(
        np.asarray(x, dtype=np.float32),
        np.asarray(w_qkv, dtype=np.float32),
        np.asarray(w_out, dtype=np.float32),
        np.asarray(b_out, dtype=np.float32),
    )
    return out
